# revision 32
# baseline (speedup 1.0000x reference)
"""Trainium2 Bass kernel for nn_LinearReg_55508157333593.

Computes: loss = (c_omega * 0.001 / N) * sum over all rows/groups of
L2 norms of 25-element groups of weight [100000, 800] f32.

Since each row is 32 contiguous groups of 25 floats and rows are contiguous,
the whole buffer is just 3.2M consecutive 25-float groups. We shard the flat
array across 8 NeuronCores (10M floats each) and stream each core's slab
through SBUF as [128, 78125] (each partition owns 3125 consecutive groups).

Raw-Bass manual pipeline (no Tile, no Block barrier), per chunk i:
  SP:  DMA chunk i into input slot i%B         (per-slot completion sems)
  ACT: square chunk i in place (SBUF->SBUF)
  DVE: per-group (25) reduce into this chunk's slice of gs_all [128, 3125]
Endgame: batched ACT sqrts over segments of gs_all (bulk segment overlaps
the stream; the last segment is tiny), each with a fused per-partition
row-sum (accum_out -> pr column), then PE matmul ones.T @ pr -> PSUM,
DVE copy to SBUF, single-partition DMA out. A dummy Sqrt is ACT's first
instruction so one ACT table load (sqrt_and_others, which also contains
square) serves the whole kernel. The host sums the 8 cores' outputs in
float64 and applies the scaling.
"""

import sys

import numpy as np

if "/opt/trn_rl_repo" not in sys.path:
    sys.path.insert(0, "/opt/trn_rl_repo")

N_CORES = 8
P = 128                      # SBUF partitions
GROUP = 25                   # elements per group
C_OMEGA = 0.001
N_ROWS = 100000
ROW = 800                    # elements per row
F_PER_PART = (N_ROWS * ROW) // (N_CORES * P)   # 78125 floats/partition/core

# chunk schedule (floats per partition; multiples of GROUP, sums to 78125):
# big chunks for streaming, finer chunks near the end (quicker input-slot
# turnaround when DVE paces), then a descending tail so the serial compute
# chain after the last DMA byte is short.
SCHEDULE = [3125] * 24 + [625] * 4 + [500, 125]
SEG_BOUNDS = [24, 29, 30]    # sqrt segments: chunks [0,24), [24,29), [29,30)
FIRST_SQRT_AFTER = 26        # emit segment-0 sqrt after this square (overlap)

_compiled = None
LAST_RESULTS = None          # BassKernelResults of the most recent run


def build(f_per_part=F_PER_PART, schedule=None, in_bufs=12, seg_bounds=None,
          first_sqrt_after=None):
    """Build and compile the per-core raw-Bass program."""
    from concourse import bacc, mybir

    if schedule is None:
        schedule = SCHEDULE
        seg_bounds = SEG_BOUNDS
        first_sqrt_after = FIRST_SQRT_AFTER
    n = len(schedule)
    if seg_bounds is None:
        seg_bounds = [max(1, n - 1), n] if n > 1 else [n]
    if first_sqrt_after is None:
        first_sqrt_after = seg_bounds[0]
    assert sum(schedule) == f_per_part
    assert all(s % GROUP == 0 for s in schedule)
    assert seg_bounds[-1] == n and sorted(seg_bounds) == seg_bounds
    assert first_sqrt_after >= seg_bounds[0] - 1
    offs = [sum(schedule[:i]) for i in range(n)]
    gpcs = [s // GROUP for s in schedule]
    goffs = [sum(gpcs[:i]) for i in range(n + 1)]
    total_g = goffs[n]
    n_segs = len(seg_bounds)
    # (end_chunk, gstart, gend) per sqrt segment
    segs = []
    prev = 0
    for b in seg_bounds:
        segs.append((b, goffs[prev], goffs[b]))
        prev = b
    max_sz = max(schedule)
    f32 = mybir.dt.float32
    Act = mybir.ActivationFunctionType

    nc = bacc.Bacc("TRN2", target_bir_lowering=False, debug=False,
                   num_devices=N_CORES)
    x = nc.dram_tensor("x", [P, f_per_part], f32, kind="ExternalInput").ap()
    # single-partition output: one small DMA descriptor, fast completion
    out = nc.dram_tensor("out", [1, n_segs], f32, kind="ExternalOutput").ap()

    B = in_bufs
    # one contiguous ring so a single DVE reduce can span several slots
    ring = nc.alloc_sbuf_tensor("ring", [P, B * max_sz], f32).ap()
    t = [ring[:, b * max_sz:(b + 1) * max_sz] for b in range(B)]

    # one reduce per chunk: grouping several chunks into one reduce op was
    # measured slower — a grouped reduce can only start after its LAST
    # square, which backloads DVE and stretches the endgame.
    red_groups = [[i] for i in range(n)]
    r_of = {i: i for i in range(n)}

    gs_all = nc.alloc_sbuf_tensor("gs_all", [P, total_g], f32).ap()
    gn = nc.alloc_sbuf_tensor("gn", [P, total_g], f32).ap()
    pr = nc.alloc_sbuf_tensor("pr", [P, n_segs], f32).ap()
    res_sb = nc.alloc_sbuf_tensor("res_sb", [1, n_segs], f32).ap()
    dm = nc.alloc_sbuf_tensor("dm_scratch", [1, 1], f32).ap()
    ps = nc.alloc_psum_tensor("ps", [1, n_segs], f32).ap()
    ones = nc.const_aps.aps[(f32, 1.0)]   # preamble-initialized [128, 1]

    dma_sems = [nc.alloc_semaphore(f"dma_sem{b}") for b in range(B)]
    out_sem = nc.alloc_semaphore("out_sem")
    sq_sem = nc.alloc_semaphore("sq_sem")       # ACT square i done
    red_sem = nc.alloc_semaphore("red_sem")     # DVE reduce i done
    sqrt_sem = nc.alloc_semaphore("sqrt_sem")   # ACT segment sqrts done
    mm_sem = nc.alloc_semaphore("mm_sem")       # PE partition-sum done
    cp_sem = nc.alloc_semaphore("cp_sem")       # PSUM->SBUF copy done

    def emit_sp(sp):
        for i in range(n):
            if i >= B:
                # input slot free once the reduce op covering it completed
                sp.wait_ge(red_sem, r_of[i - B] + 1)
            sp.dma_start(
                t[i % B][:, :schedule[i]], x[:, offs[i]:offs[i] + schedule[i]]
            ).then_inc(dma_sems[i % B], 16)
        sp.wait_ge(cp_sem, 1)
        sp.dma_start(out, res_sb).then_inc(out_sem, 16)
        sp.wait_ge(out_sem, 16)

    def emit_act(act):
        # table prefetch: first activation is a Sqrt, so the one table set
        # loaded (sqrt_and_others) also covers Square -> no mid-kernel load
        act.activation(dm, ones[0:1, :], Act.Sqrt)

        def emit_seg(s):
            end_chunk, glo, ghi = segs[s]
            act.wait_ge(red_sem, r_of[end_chunk - 1] + 1)
            act.activation(gn[:, glo:ghi], gs_all[:, glo:ghi], Act.Sqrt,
                           accum_out=pr[:, s:s + 1]).then_inc(sqrt_sem, 1)

        emitted = 0
        for i in range(n):
            act.wait_ge(dma_sems[i % B], 16 * (i // B + 1))
            act.activation(t[i % B][:, :schedule[i]], t[i % B][:, :schedule[i]],
                           Act.Square).then_inc(sq_sem, 1)
            if emitted == 0 and i >= first_sqrt_after and n_segs > 1:
                emit_seg(0)
                emitted = 1
        for s in range(emitted, n_segs):
            emit_seg(s)

    def emit_dve(dve):
        for g in red_groups:
            c0, c1 = g[0], g[-1]
            dve.wait_ge(sq_sem, c1 + 1)
            lo = (c0 % B) * max_sz
            span = sum(schedule[c] for c in g)
            dve.reduce_sum(
                gs_all[:, goffs[c0]:goffs[c1 + 1]],
                ring[:, lo:lo + span].rearrange("p (g k) -> p g k", k=GROUP),
                axis=mybir.AxisListType.X,
            ).then_inc(red_sem, 1)
        dve.wait_ge(mm_sem, 1)
        dve.tensor_copy(res_sb, ps).then_inc(cp_sem, 1)

    def emit_pe(pe):
        pe.wait_ge(sqrt_sem, n_segs)
        pe.matmul(ps, ones, pr, start=True, stop=True).then_inc(mm_sem, 1)

    emit_sp(nc.sync)
    emit_act(nc.scalar)
    emit_dve(nc.vector)
    emit_pe(nc.tensor)

    nc.compile()
    return nc


def kernel(weight, c_omega):
    global _compiled, LAST_RESULTS
    from concourse.bass_utils import run_bass_kernel_spmd

    if _compiled is None:
        _compiled = build()
    nc = _compiled

    w = np.asarray(weight)
    if w.dtype != np.float32:
        w = w.astype(np.float32)
    w = np.ascontiguousarray(w)
    flat = w.reshape(-1)
    per_core = flat.size // N_CORES
    in_maps = [
        {"x": flat[c * per_core:(c + 1) * per_core].reshape(P, F_PER_PART)}
        for c in range(N_CORES)
    ]
    LAST_RESULTS = run_bass_kernel_spmd(nc, in_maps,
                                        core_ids=list(range(N_CORES)))
    total = 0.0
    for r in LAST_RESULTS.results:
        total += float(r["out"].astype(np.float64).sum())
    loss = total / N_ROWS * (C_OMEGA * float(c_omega))
    return np.float32(loss)


def selftest_sim(f_per_part=625, schedule=(250, 250, 75, 25, 25),
                 in_bufs=3, seed=0, **kw):
    """CoreSim check on a scaled-down instance; returns max rel err."""
    from concourse.bass_interp import CoreSim

    nc = build(f_per_part=f_per_part, schedule=list(schedule),
               in_bufs=in_bufs, **kw)
    rng = np.random.default_rng(seed)
    xv = rng.standard_normal((P, f_per_part)).astype(np.float32)
    sim = CoreSim(nc)
    sim.tensor("x")[:] = xv
    sim.simulate()
    got = float(np.array(sim.tensor("out")).astype(np.float64).sum())
    g = xv.reshape(P, f_per_part // GROUP, GROUP)
    want = float(np.sqrt((g.astype(np.float64) ** 2).sum(-1)).sum())
    return abs(got - want) / abs(want)


# revision 39
# speedup vs baseline: 1.0757x; 1.0757x over previous
"""Trainium2 Bass kernel for nn_LinearReg_55508157333593.

Computes: loss = (c_omega * 0.001 / N) * sum over all rows/groups of
L2 norms of 25-element groups of weight [100000, 800] f32.

Since each row is 32 contiguous groups of 25 floats and rows are contiguous,
the whole buffer is just 3.2M consecutive 25-float groups. We shard the flat
array across 8 NeuronCores (10M floats each) and stream each core's slab
through SBUF as [128, 78125] (each partition owns 3125 consecutive groups).

Raw-Bass manual pipeline (no Tile, no Block barrier), per chunk i:
  SP:  DMA chunk i into input slot i%B         (per-slot completion sems)
  ACT: square chunk i in place (SBUF->SBUF)
  DVE: per-group (25) reduce into this chunk's slice of gs_all [128, 3125]
Endgame: batched ACT sqrts over segments of gs_all (bulk segment overlaps
the stream; the last segment is tiny), each with a fused per-partition
row-sum (accum_out -> pr column), then PE matmul ones.T @ pr -> PSUM,
DVE copy to SBUF, single-partition DMA out. A dummy Sqrt is ACT's first
instruction so one ACT table load (sqrt_and_others, which also contains
square) serves the whole kernel. The host sums the 8 cores' outputs in
float64 and applies the scaling.
"""

import sys

import numpy as np

if "/opt/trn_rl_repo" not in sys.path:
    sys.path.insert(0, "/opt/trn_rl_repo")

N_CORES = 8
P = 128                      # SBUF partitions
GROUP = 25                   # elements per group
C_OMEGA = 0.001
N_ROWS = 100000
ROW = 800                    # elements per row
F_PER_PART = (N_ROWS * ROW) // (N_CORES * P)   # 78125 floats/partition/core

# chunk schedule (floats per partition; multiples of GROUP, sums to 78125):
# big chunks for streaming, finer chunks near the end (quicker input-slot
# turnaround when DVE paces), then a descending tail so the serial compute
# chain after the last DMA byte is short.
SCHEDULE = [3125] * 24 + [625] * 4 + [500, 125]
SEG_BOUNDS = [24, 29, 30]    # sqrt segments: chunks [0,24), [24,29), [29,30)
FIRST_SQRT_AFTER = 26        # emit segment-0 sqrt after this square (overlap)

_compiled = None
LAST_RESULTS = None          # BassKernelResults of the most recent run


def build(f_per_part=F_PER_PART, schedule=None, in_bufs=12, seg_bounds=None,
          first_sqrt_after=None):
    """Build and compile the per-core raw-Bass program."""
    from concourse import bacc, mybir

    if schedule is None:
        schedule = SCHEDULE
        seg_bounds = SEG_BOUNDS
        first_sqrt_after = FIRST_SQRT_AFTER
    n = len(schedule)
    if seg_bounds is None:
        seg_bounds = [max(1, n - 1), n] if n > 1 else [n]
    if first_sqrt_after is None:
        first_sqrt_after = seg_bounds[0]
    assert sum(schedule) == f_per_part
    assert all(s % GROUP == 0 for s in schedule)
    assert seg_bounds[-1] == n and sorted(seg_bounds) == seg_bounds
    assert first_sqrt_after >= seg_bounds[0] - 1
    offs = [sum(schedule[:i]) for i in range(n)]
    gpcs = [s // GROUP for s in schedule]
    goffs = [sum(gpcs[:i]) for i in range(n + 1)]
    total_g = goffs[n]
    n_segs = len(seg_bounds)
    # (end_chunk, gstart, gend) per sqrt segment
    segs = []
    prev = 0
    for b in seg_bounds:
        segs.append((b, goffs[prev], goffs[b]))
        prev = b
    max_sz = max(schedule)
    f32 = mybir.dt.float32
    Act = mybir.ActivationFunctionType

    nc = bacc.Bacc("TRN2", target_bir_lowering=False, debug=False,
                   num_devices=N_CORES)
    x = nc.dram_tensor("x", [P, f_per_part], f32, kind="ExternalInput").ap()
    # single-partition output: one small DMA descriptor, fast completion
    out = nc.dram_tensor("out", [1, n_segs], f32, kind="ExternalOutput").ap()

    B = in_bufs
    # one contiguous ring so a single DVE reduce can span several slots
    ring = nc.alloc_sbuf_tensor("ring", [P, B * max_sz], f32).ap()
    t = [ring[:, b * max_sz:(b + 1) * max_sz] for b in range(B)]

    # one square+reduce PIECE per chunk, except the first two chunks are
    # split in half so DVE's pipeline wakes up earlier (its first wait is
    # released by a half-size square instead of a full one). Grouping
    # several chunks into one reduce was measured slower (backloads DVE).
    pieces = []                  # (chunk, lo, hi) in floats, lo/hi % 25 == 0
    for i in range(n):
        sz = schedule[i]
        if i < 2 and sz >= 2 * GROUP:
            half = (sz // 2 // GROUP) * GROUP
            pieces.append((i, 0, half))
            pieces.append((i, half, sz))
        else:
            pieces.append((i, 0, sz))
    last_piece = {}              # chunk -> index of its last piece
    for p, (c, _, _) in enumerate(pieces):
        last_piece[c] = p
    r_of = last_piece            # reduce ops mirror pieces 1:1

    gs_all = nc.alloc_sbuf_tensor("gs_all", [P, total_g], f32).ap()
    gn = nc.alloc_sbuf_tensor("gn", [P, total_g], f32).ap()
    pr = nc.alloc_sbuf_tensor("pr", [P, n_segs], f32).ap()
    res_sb = nc.alloc_sbuf_tensor("res_sb", [1, n_segs], f32).ap()
    dm = nc.alloc_sbuf_tensor("dm_scratch", [1, 1], f32).ap()
    ps = nc.alloc_psum_tensor("ps", [1, n_segs], f32).ap()
    ones = nc.const_aps.aps[(f32, 1.0)]   # preamble-initialized [128, 1]

    dma_sems = [nc.alloc_semaphore(f"dma_sem{b}") for b in range(B)]
    out_sem = nc.alloc_semaphore("out_sem")
    sq_sem = nc.alloc_semaphore("sq_sem")       # ACT square i done
    red_sem = nc.alloc_semaphore("red_sem")     # DVE reduce i done
    sqrt_sem = nc.alloc_semaphore("sqrt_sem")   # ACT segment sqrts done
    mm_sem = nc.alloc_semaphore("mm_sem")       # PE partition-sum done
    cp_sem = nc.alloc_semaphore("cp_sem")       # PSUM->SBUF copy done

    def emit_sp(sp):
        for i in range(n):
            if i >= B:
                # input slot free once the reduce op covering it completed
                sp.wait_ge(red_sem, r_of[i - B] + 1)
            sp.dma_start(
                t[i % B][:, :schedule[i]], x[:, offs[i]:offs[i] + schedule[i]]
            ).then_inc(dma_sems[i % B], 16)
        sp.wait_ge(cp_sem, 1)
        sp.dma_start(out, res_sb).then_inc(out_sem, 16)
        sp.wait_ge(out_sem, 16)

    def emit_act(act):
        # table prefetch: first activation is a Sqrt, so the one table set
        # loaded (sqrt_and_others) also covers Square -> no mid-kernel load
        act.activation(dm, ones[0:1, :], Act.Sqrt)

        def emit_seg(s):
            end_chunk, glo, ghi = segs[s]
            act.wait_ge(red_sem, r_of[end_chunk - 1] + 1)
            act.activation(gn[:, glo:ghi], gs_all[:, glo:ghi], Act.Sqrt,
                           accum_out=pr[:, s:s + 1]).then_inc(sqrt_sem, 1)

        emitted = 0
        prev_chunk = -1
        for c, lo, hi in pieces:
            if c != prev_chunk:
                if (emitted == 0 and prev_chunk >= first_sqrt_after
                        and n_segs > 1):
                    emit_seg(0)
                    emitted = 1
                act.wait_ge(dma_sems[c % B], 16 * (c // B + 1))
                prev_chunk = c
            act.activation(t[c % B][:, lo:hi], t[c % B][:, lo:hi],
                           Act.Square).then_inc(sq_sem, 1)
        for s in range(emitted, n_segs):
            emit_seg(s)

    def emit_dve(dve):
        for p, (c, lo, hi) in enumerate(pieces):
            dve.wait_ge(sq_sem, p + 1)
            base = (c % B) * max_sz
            dve.reduce_sum(
                gs_all[:, goffs[c] + lo // GROUP:goffs[c] + hi // GROUP],
                ring[:, base + lo:base + hi].rearrange("p (g k) -> p g k",
                                                       k=GROUP),
                axis=mybir.AxisListType.X,
            ).then_inc(red_sem, 1)
        dve.wait_ge(mm_sem, 1)
        dve.tensor_copy(res_sb, ps).then_inc(cp_sem, 1)

    def emit_pe(pe):
        pe.wait_ge(sqrt_sem, n_segs)
        pe.matmul(ps, ones, pr, start=True, stop=True).then_inc(mm_sem, 1)

    emit_sp(nc.sync)
    emit_act(nc.scalar)
    emit_dve(nc.vector)
    emit_pe(nc.tensor)

    nc.compile()
    return nc


def kernel(weight, c_omega):
    global _compiled, LAST_RESULTS
    from concourse.bass_utils import run_bass_kernel_spmd

    if _compiled is None:
        _compiled = build()
    nc = _compiled

    w = np.asarray(weight)
    if w.dtype != np.float32:
        w = w.astype(np.float32)
    w = np.ascontiguousarray(w)
    flat = w.reshape(-1)
    per_core = flat.size // N_CORES
    in_maps = [
        {"x": flat[c * per_core:(c + 1) * per_core].reshape(P, F_PER_PART)}
        for c in range(N_CORES)
    ]
    LAST_RESULTS = run_bass_kernel_spmd(nc, in_maps,
                                        core_ids=list(range(N_CORES)))
    total = 0.0
    for r in LAST_RESULTS.results:
        total += float(r["out"].astype(np.float64).sum())
    loss = total / N_ROWS * (C_OMEGA * float(c_omega))
    return np.float32(loss)


def selftest_sim(f_per_part=625, schedule=(250, 250, 75, 25, 25),
                 in_bufs=3, seed=0, **kw):
    """CoreSim check on a scaled-down instance; returns max rel err."""
    from concourse.bass_interp import CoreSim

    nc = build(f_per_part=f_per_part, schedule=list(schedule),
               in_bufs=in_bufs, **kw)
    rng = np.random.default_rng(seed)
    xv = rng.standard_normal((P, f_per_part)).astype(np.float32)
    sim = CoreSim(nc)
    sim.tensor("x")[:] = xv
    sim.simulate()
    got = float(np.array(sim.tensor("out")).astype(np.float64).sum())
    g = xv.reshape(P, f_per_part // GROUP, GROUP)
    want = float(np.sqrt((g.astype(np.float64) ** 2).sum(-1)).sum())
    return abs(got - want) / abs(want)
